# revision 7
# baseline (speedup 1.0000x reference)
"""EntityGuidedCrossAttention TRN2 kernel (8 NeuronCores, data-parallel over classes).

Math restructure (exact): labels are contiguous per class, so attention is
block-diagonal.  With folded weights (host-side, weights-only algebra):
    Wqk = Wq^T Wk,  bqk = bq Wk          ->  Qk = ent @ Wqk + bqk
    Wvo = Wv^T Wo^T, bvo = bv Wo^T + bo  ->  OUT = pooled @ Wvo + bvo
    score[c,k] = Qk[c] . sup[c*K+k] / sqrt(D)   (bk is softmax-shift-invariant)
    pooled[c]  = sum_k softmax_w[c,k] * sup[c*K+k]
    res        = sup + OUT[class(row)]

v3 (evidence from v1/v2 traces): PE is the wall -- the HW duty-throttles
the tensor engine to ~1.1-1.2GHz effective whenever it is continuously
busy, so PE cycles are ~2x their nominal cost.  Changes:
  - B's onehot broadcast (ind.T @ Qk) runs as an fp8e4 DoubleRow matmul
    (0.5 cycles/row): the 0/1 indicator is exact in fp8, Qk is quantized
    to fp8 (the induced ~3% logit jitter averages out through softmax ->
    ~6e-3 final rel err on top of 5e-3 bf16 base; gate is 2e-2).  A is
    computed as two 32-class chains so both Qk row-halves land on
    partitions 0-31, which DoubleRow's 2-k-tile layout requires.
  - D(g-1) matmuls are interleaved between B(g) matmuls in PE program
    order (v1/v2 ran B-block then D-block per group, ping-ponging PE and
    DVE at ~50% each).
  - Score dots read qkb straight from PSUM on DVE (a bf16 staging drain
    buys nothing: the 3-op stt has no DVE fast modes).
  - Softmax normalizer r: ONE matmul over a densely packed weight bank
    (v1 used 32 single-column matmuls).
  - E is batched over all 64 classes (v2's per-half E doubled its cost).
  - F consumers split across engines: 1/4 DVE-fused from PSUM, 1/2
    ACT-drain + DVE 2x add, 1/4 ACT-drain + GPSIMD add; res tiles DMA
    out per pair so the output stream rides under F.
"""

import numpy as np

N_CLASSES = 512
K_SHOTS = 64
D = 1024
NK = N_CLASSES * K_SHOTS
N_CORES = 8
C_LOC = N_CLASSES // N_CORES          # 64 classes per core
R_LOC = NK // N_CORES                 # 4096 support rows per core
P = 128
TILES = R_LOC // P                    # 32 row-tiles of 128
DCH = D // P                          # 8 contraction chunks
GSZ = 8                               # tiles per softmax group
GROUPS = TILES // GSZ                 # 4
INV_SQRT_D = 1.0 / float(np.sqrt(D))
WPAD = 66                             # w_all per-tile column pitch (64 + 2)

USE_FP8_B = True                      # fp8 DoubleRow for the B broadcast

_NC_CACHE = None


def _build_nc():
    import concourse.bacc as bacc
    import concourse.tile as tile
    import concourse.mybir as mybir
    from concourse.masks import make_identity

    f32 = mybir.dt.float32
    bf16 = mybir.dt.bfloat16
    fp8 = mybir.dt.float8e4
    ADD = mybir.AluOpType.add
    MUL = mybir.AluOpType.mult
    EXP = mybir.ActivationFunctionType.Exp
    CPY = mybir.ActivationFunctionType.Copy
    DROW = mybir.MatmulPerfMode.DoubleRow

    nc = bacc.Bacc("TRN2", target_bir_lowering=False, debug=False,
                   num_devices=N_CORES)

    sup_d = nc.dram_tensor("sup", [R_LOC, D], bf16, kind="ExternalInput").ap()
    entt_d = nc.dram_tensor("entt", [D, C_LOC], bf16, kind="ExternalInput").ap()
    if USE_FP8_B:
        # ind8[p, i, r] = 1 iff label_loc[r] == 32*i + p  (2 k-tiles of 32)
        ind_d = nc.dram_tensor("ind", [C_LOC // 2, 2 * R_LOC], fp8,
                               kind="ExternalInput").ap()
    else:
        ind_d = nc.dram_tensor("ind", [C_LOC, R_LOC], bf16,
                               kind="ExternalInput").ap()
    # F's broadcast always needs the bf16 indicator
    indf_d = nc.dram_tensor("indf", [C_LOC, R_LOC], bf16,
                            kind="ExternalInput").ap()
    wqk_d = nc.dram_tensor("wqk", [D, D], bf16, kind="ExternalInput").ap()
    wvo_d = nc.dram_tensor("wvo", [D, D], bf16, kind="ExternalInput").ap()
    bqk_d = nc.dram_tensor("bqk", [1, D], bf16, kind="ExternalInput").ap()
    bvo_d = nc.dram_tensor("bvo", [1, D], bf16, kind="ExternalInput").ap()
    res_d = nc.dram_tensor("res", [R_LOC, D], bf16, kind="ExternalOutput").ap()

    with tile.TileContext(nc) as tc:
        with (
            tc.tile_pool(name="const", bufs=1) as const,
            tc.tile_pool(name="sbB", bufs=2) as sbB,
            tc.tile_pool(name="psQ", bufs=2, space="PSUM") as psQ,
            tc.tile_pool(name="psP", bufs=1, space="PSUM") as psP,
            tc.tile_pool(name="psS", bufs=1, space="PSUM") as psS,
            tc.tile_pool(name="psT", bufs=1, space="PSUM") as psT,
        ):
            idf = const.tile([P, P], f32)
            make_identity(nc, idf)
            idb = const.tile([P, P], bf16)
            nc.scalar.copy(out=idb, in_=idf)
            ones_b = const.tile([1, C_LOC], bf16)
            nc.vector.memset(ones_b, 1.0)
            ones_col = const.tile([P, 1], bf16)
            nc.vector.memset(ones_col, 1.0)

            entt_sb = const.tile([P, DCH * C_LOC], bf16)
            if USE_FP8_B:
                ind_sb = const.tile([C_LOC // 2, 2 * R_LOC], fp8)
                qk8_sb = const.tile([C_LOC // 2, 2 * D], fp8)
            else:
                ind_sb = const.tile([C_LOC, R_LOC], bf16)
                qk_sb = const.tile([C_LOC, D], bf16)
            indf_sb = const.tile([C_LOC, R_LOC], bf16)
            wqk_sb = const.tile([P, DCH * D], bf16)
            wvo_sb = const.tile([P, DCH * D], bf16)
            bqk_sb = const.tile([1, D], bf16)
            bvo_sb = const.tile([1, D], bf16)
            out_sb = const.tile([C_LOC, D], bf16)
            pooled_sb = const.tile([C_LOC, D], bf16)
            pooledt_sb = const.tile([P, DCH * C_LOC], bf16)
            sup_all = const.tile([P, TILES * D], bf16)
            ri_sb = const.tile([C_LOC, 1], f32)
            # padded softmax-weight lhsT bank: tile t's two columns live at
            # WPAD*t (+1); its D-matmul lhsT window is [64t, 64t+64) -- only
            # tile t's own pair lands inside its window.
            w_all = const.tile([P, WPAD * TILES], bf16)
            nc.vector.memset(w_all, 0.0)
            # densely packed copy (col 2t, 2t+1) for the single normalizer
            # matmul r = w_r^T @ ones.
            w_r = const.tile([P, C_LOC], bf16)
            nc.vector.memset(w_r, 0.0)

            # ---------------- input DMAs (one FIFO queue, issue order =
            # arrival order): consts -> wqk -> ind -> sup -> wvo ------------
            nc.sync.dma_start(out=bqk_sb, in_=bqk_d)
            nc.sync.dma_start(out=bvo_sb, in_=bvo_d)
            nc.sync.dma_start(
                out=entt_sb.rearrange("p (ch c) -> p ch c", ch=DCH),
                in_=entt_d.rearrange("(ch p) c -> p ch c", p=P),
            )
            wqk_v = wqk_sb.rearrange("p (ch d) -> p ch d", ch=DCH)
            wqkd_v = wqk_d.rearrange("(ch p) d -> p ch d", p=P)
            for h4 in range(4):
                nc.sync.dma_start(out=wqk_v[:, 2 * h4:2 * h4 + 2, :],
                                  in_=wqkd_v[:, 2 * h4:2 * h4 + 2, :])
            nc.sync.dma_start(out=ind_sb, in_=ind_d)
            nc.sync.dma_start(out=indf_sb, in_=indf_d)
            sup_v = sup_all.rearrange("p (t d) -> p t d", d=D)
            supd_v = sup_d.rearrange("(t p) d -> p t d", p=P)
            for k in range(16):
                nc.sync.dma_start(out=sup_v[:, 2 * k:2 * k + 2, :],
                                  in_=supd_v[:, 2 * k:2 * k + 2, :])
            wvo_v = wvo_sb.rearrange("p (ch d) -> p ch d", ch=DCH)
            wvod_v = wvo_d.rearrange("(ch p) d -> p ch d", p=P)
            for h4 in range(4):
                nc.sync.dma_start(out=wvo_v[:, 2 * h4:2 * h4 + 2, :],
                                  in_=wvod_v[:, 2 * h4:2 * h4 + 2, :])

            # ---------------- PE warmup (short: p-state ramp) --------------
            with nc.named_scope("warmup"):
                for _ in range(4):
                    w_ps = psQ.tile([P, D], f32, tag="ring")
                    nc.tensor.transpose(w_ps[:, 0:P], idf, idf)

            # ---------------- Phase A: Qk = entT.T @ Wqk + bqk -------------
            with nc.named_scope("phaseA"):
                if USE_FP8_B:
                    # two 32-class chains so both row-halves of Qk land on
                    # partitions 0-31 (DoubleRow k-tiles share partitions)
                    for half in range(2):
                        q_ps = psQ.tile([P, D], f32, tag="ring")
                        for ch in range(DCH):
                            for nh in range(2):
                                nc.tensor.matmul(
                                    q_ps[0:32, nh * 512:(nh + 1) * 512],
                                    entt_sb[:, ch * C_LOC + 32 * half:
                                            ch * C_LOC + 32 * half + 32],
                                    wqk_sb[:, ch * D + nh * 512:
                                           ch * D + (nh + 1) * 512],
                                    start=(ch == 0), stop=False,
                                )
                        for nh in range(2):
                            nc.tensor.matmul(
                                q_ps[0:32, nh * 512:(nh + 1) * 512],
                                ones_b[0:1, 0:32],
                                bqk_sb[0:1, nh * 512:(nh + 1) * 512],
                                start=False, stop=True,
                            )
                        nc.scalar.copy(
                            out=qk8_sb[:, half * D:(half + 1) * D],
                            in_=q_ps[0:32, :])
                else:
                    q_ps = psQ.tile([P, D], f32, tag="ring")
                    for ch in range(DCH):
                        for nh in range(2):
                            nc.tensor.matmul(
                                q_ps[0:C_LOC, nh * 512:(nh + 1) * 512],
                                entt_sb[:, ch * C_LOC:(ch + 1) * C_LOC],
                                wqk_sb[:, ch * D + nh * 512:
                                       ch * D + (nh + 1) * 512],
                                start=(ch == 0), stop=False,
                            )
                    for nh in range(2):
                        nc.tensor.matmul(
                            q_ps[0:C_LOC, nh * 512:(nh + 1) * 512],
                            ones_b, bqk_sb[0:1, nh * 512:(nh + 1) * 512],
                            start=False, stop=True,
                        )
                    nc.scalar.copy(out=qk_sb, in_=q_ps[0:C_LOC, :])

            res_v = res_d.rearrange("(t p) d -> p t d", p=P)
            prod = sbB.tile([P, D], bf16, tag="prod", bufs=1)
            if USE_FP8_B:
                ind_v = ind_sb.rearrange("p (i r) -> p i r", i=2)
                qk8_v = qk8_sb.rearrange("p (i d) -> p i d", i=2)

            def b_tile(t, s8, j):
                """scores for tile t -> s8[:, j] (exp/normalize later)."""
                qkb = psQ.tile([P, D], f32, tag="ring")
                if USE_FP8_B:
                    for nh in range(2):
                        nc.tensor.matmul(
                            qkb[:, nh * 512:(nh + 1) * 512],
                            ind_v[:, :, t * P:(t + 1) * P],
                            qk8_v[:, :, nh * 512:(nh + 1) * 512],
                            start=True, stop=True,
                            perf_mode=DROW,
                        )
                else:
                    for nh in range(2):
                        nc.tensor.matmul(
                            qkb[:, nh * 512:(nh + 1) * 512],
                            ind_sb[:, t * P:(t + 1) * P],
                            qk_sb[:, nh * 512:(nh + 1) * 512],
                            start=True, stop=True,
                        )
                # DVE reads the PSUM qkb directly (no staging drain)
                nc.vector.scalar_tensor_tensor(
                    out=prod, in0=qkb, scalar=INV_SQRT_D,
                    in1=sup_all[:, t * D:(t + 1) * D],
                    op0=MUL, op1=MUL, accum_out=s8[:, j:j + 1])

            def c_group(g, s8):
                """exp + scatter the weight pairs into w_all / w_r."""
                e8 = sbB.tile([P, GSZ], bf16, tag="e8", bufs=2)
                nc.scalar.activation(out=e8, in_=s8, func=EXP)
                b_all = WPAD * GSZ * g
                b_r = 2 * GSZ * g
                nc.vector.tensor_copy(
                    out=w_all[0:K_SHOTS, b_all:b_all + WPAD * GSZ:WPAD],
                    in_=e8[0:K_SHOTS, :])
                nc.vector.tensor_copy(
                    out=w_all[K_SHOTS:P,
                              b_all + 1:b_all + WPAD * (GSZ - 1) + 2:WPAD],
                    in_=e8[K_SHOTS:P, :])
                nc.vector.tensor_copy(
                    out=w_r[0:K_SHOTS, b_r:b_r + 2 * GSZ:2],
                    in_=e8[0:K_SHOTS, :])
                nc.vector.tensor_copy(
                    out=w_r[K_SHOTS:P, b_r + 1:b_r + 2 * GSZ:2],
                    in_=e8[K_SHOTS:P, :])

            def d_tile(t, pooled_ps):
                """pooled += w_tile.T @ sup_tile (chained accumulation)."""
                for nh in range(2):
                    nc.tensor.matmul(
                        pooled_ps[:, nh * 512:(nh + 1) * 512],
                        w_all[:, 64 * t:64 * t + 64],
                        sup_all[:, t * D + nh * 512:t * D + (nh + 1) * 512],
                        start=(t == 0), stop=(t == TILES - 1),
                    )

            # ------------- B/C/D software-pipelined per tile ---------------
            # PE order: [B(g,0), D(g-1,0), B(g,1), D(g-1,1), ...] so the PE
            # keeps feeding DVE fresh qkb tiles while it chews D matmuls.
            pooled_ps = psP.tile([C_LOC, D], f32)
            r_ps = psS.tile([C_LOC, 1], f32)
            s8s = []
            with nc.named_scope("phaseBCD"):
                for g in range(GROUPS):
                    s8 = sbB.tile([P, GSZ], f32, tag="s8", bufs=2)
                    s8s.append(s8)
                    for j in range(GSZ):
                        b_tile(g * GSZ + j, s8, j)
                        if g > 0:
                            d_tile((g - 1) * GSZ + j, pooled_ps)
                    c_group(g, s8)
                for j in range(GSZ):
                    d_tile((GROUPS - 1) * GSZ + j, pooled_ps)
                nc.tensor.matmul(r_ps, w_r, ones_col, start=True, stop=True)

            # ---------------- Phase E: OUT = (pooled/r) @ Wvo + bvo --------
            with nc.named_scope("phaseE"):
                nc.vector.reciprocal(ri_sb, r_ps)
                nc.scalar.activation(out=pooled_sb, in_=pooled_ps,
                                     func=CPY, scale=ri_sb[:, 0:1])
                for ch in range(DCH):
                    tp = psT.tile([P, C_LOC], bf16, tag="tp")
                    nc.tensor.transpose(
                        tp, pooled_sb[:, ch * P:(ch + 1) * P],
                        idb[0:C_LOC, 0:C_LOC],
                    )
                    nc.scalar.copy(
                        out=pooledt_sb[:, ch * C_LOC:(ch + 1) * C_LOC],
                        in_=tp,
                    )
                o_ps = psQ.tile([P, D], f32, tag="ring")
                for ch in range(DCH):
                    for nh in range(2):
                        nc.tensor.matmul(
                            o_ps[0:C_LOC, nh * 512:(nh + 1) * 512],
                            pooledt_sb[:, ch * C_LOC:(ch + 1) * C_LOC],
                            wvo_sb[:, ch * D + nh * 512:ch * D + (nh + 1) * 512],
                            start=(ch == 0), stop=False,
                        )
                for nh in range(2):
                    nc.tensor.matmul(
                        o_ps[0:C_LOC, nh * 512:(nh + 1) * 512],
                        ones_b, bvo_sb[0:1, nh * 512:(nh + 1) * 512],
                        start=False, stop=True,
                    )
                nc.scalar.copy(out=out_sb, in_=o_ps[0:C_LOC, :])

            # ---------------- Phase F: res = sup + OUT[class(row)] ---------
            with nc.named_scope("phaseF"):
                for t in range(TILES):
                    ob = psQ.tile([P, D], f32, tag="ring")
                    for nh in range(2):
                        nc.tensor.matmul(
                            ob[:, nh * 512:(nh + 1) * 512],
                            indf_sb[:, t * P:(t + 1) * P],
                            out_sb[:, nh * 512:(nh + 1) * 512],
                            start=True, stop=True,
                        )
                    st = sup_all[:, t * D:(t + 1) * D]
                    if t % 4 == 0:
                        # DVE adds straight from PSUM (fused drain+add)
                        nc.vector.scalar_tensor_tensor(
                            out=st, in0=ob, scalar=1.0, in1=st,
                            op0=MUL, op1=ADD)
                    else:
                        ob_sb = sbB.tile([P, D], bf16, tag="ob_sb", bufs=2)
                        nc.scalar.copy(out=ob_sb, in_=ob)
                        if t % 2 == 1:
                            nc.vector.tensor_tensor(out=st, in0=st, in1=ob_sb,
                                                    op=ADD)
                        else:
                            nc.gpsimd.tensor_tensor(out=st, in0=st, in1=ob_sb,
                                                    op=ADD)
                    if t % 2 == 1:
                        nc.sync.dma_start(
                            out=res_v[:, t - 1:t + 1, :],
                            in_=sup_v[:, t - 1:t + 1, :],
                        )

    nc.compile()
    return nc


def _get_nc():
    global _NC_CACHE
    if _NC_CACHE is None:
        _NC_CACHE = _build_nc()
    return _NC_CACHE


def _prep_in_maps(support_features, entity_vectors, support_labels,
                  Wq, bq, Wk, bk, Wv, bv, Wo, bo):
    from ml_dtypes import bfloat16, float8_e4m3fn

    sup = np.asarray(support_features, dtype=np.float32)
    ent = np.asarray(entity_vectors, dtype=np.float32)
    labels = np.asarray(support_labels, dtype=np.int32)
    wq = np.asarray(Wq, dtype=np.float32)
    wk = np.asarray(Wk, dtype=np.float32)
    wv = np.asarray(Wv, dtype=np.float32)
    wo = np.asarray(Wo, dtype=np.float32)
    bq_ = np.asarray(bq, dtype=np.float32).reshape(1, D)
    bv_ = np.asarray(bv, dtype=np.float32).reshape(1, D)
    bo_ = np.asarray(bo, dtype=np.float32).reshape(1, D)
    # bk is dropped: it adds a per-class constant to each softmax row.

    # weights-only folding (reparameterization; activation math is on-device)
    wqk = np.ascontiguousarray(wq.T @ wk).astype(bfloat16)
    wvo = np.ascontiguousarray(wv.T @ wo.T).astype(bfloat16)
    bqk = (bq_ @ wk).astype(bfloat16)
    bvo = (bv_ @ wo.T + bo_).astype(bfloat16)

    expected = np.arange(NK, dtype=np.int32) // K_SHOTS
    assert np.array_equal(labels, expected), (
        "kernel assumes exactly K_SHOTS contiguous samples per class "
        "(labels == arange(NK)//K_SHOTS)"
    )

    sup_bf = sup.astype(bfloat16)
    in_maps = []
    for c in range(N_CORES):
        lab_loc = labels[c * R_LOC:(c + 1) * R_LOC] - c * C_LOC
        indf = (lab_loc[None, :] ==
                np.arange(C_LOC, dtype=np.int32)[:, None]).astype(bfloat16)
        if USE_FP8_B:
            # [32, 2, R_LOC]: k-tile i holds classes 32i..32i+31
            ind8 = (lab_loc[None, None, :] ==
                    (np.arange(C_LOC, dtype=np.int32)
                     .reshape(2, 32).transpose(1, 0)[:, :, None])
                    ).astype(float8_e4m3fn)
            ind = np.ascontiguousarray(ind8.reshape(32, 2 * R_LOC))
        else:
            ind = indf
        in_maps.append({
            "sup": np.ascontiguousarray(sup_bf[c * R_LOC:(c + 1) * R_LOC]),
            "entt": np.ascontiguousarray(
                ent[c * C_LOC:(c + 1) * C_LOC].T).astype(bfloat16),
            "ind": np.ascontiguousarray(ind),
            "indf": np.ascontiguousarray(indf),
            "wqk": wqk, "wvo": wvo, "bqk": bqk, "bvo": bvo,
        })
    return in_maps


def _run(in_maps, **kwargs):
    from concourse.bass_utils import run_bass_kernel_spmd
    nc = _get_nc()
    return run_bass_kernel_spmd(nc, in_maps, core_ids=list(range(N_CORES)),
                                **kwargs)


def kernel(support_features, entity_vectors, support_labels,
           Wq, bq, Wk, bk, Wv, bv, Wo, bo):
    in_maps = _prep_in_maps(support_features, entity_vectors, support_labels,
                            Wq, bq, Wk, bk, Wv, bv, Wo, bo)
    r = _run(in_maps)
    return np.concatenate(
        [np.asarray(r.results[c]["res"], dtype=np.float32)
         for c in range(N_CORES)], axis=0)


# revision 8
# speedup vs baseline: 1.1206x; 1.1206x over previous
"""EntityGuidedCrossAttention TRN2 kernel (8 NeuronCores, data-parallel over classes).

Math restructure (exact): labels are contiguous per class, so attention is
block-diagonal.  With folded weights (host-side, weights-only algebra):
    Wqk = Wq^T Wk,  bqk = bq Wk          ->  Qk = ent @ Wqk + bqk
    Wvo = Wv^T Wo^T, bvo = bv Wo^T + bo  ->  OUT = pooled @ Wvo + bvo
    score[c,k] = Qk[c] . sup[c*K+k] / sqrt(D)   (bk is softmax-shift-invariant)
    pooled[c]  = sum_k softmax_w[c,k] * sup[c*K+k]
    res        = sup + OUT[class(row)]

v3 (evidence from v1/v2 traces): PE is the wall -- the HW duty-throttles
the tensor engine to ~1.1-1.2GHz effective whenever it is continuously
busy, so PE cycles are ~2x their nominal cost.  Changes:
  - B's onehot broadcast (ind.T @ Qk) runs as an fp8e4 DoubleRow matmul
    (0.5 cycles/row): the 0/1 indicator is exact in fp8, Qk is quantized
    to fp8 (the induced ~3% logit jitter averages out through softmax ->
    ~6e-3 final rel err on top of 5e-3 bf16 base; gate is 2e-2).  A is
    computed as two 32-class chains so both Qk row-halves land on
    partitions 0-31, which DoubleRow's 2-k-tile layout requires.
  - D(g-1) matmuls are interleaved between B(g) matmuls in PE program
    order (v1/v2 ran B-block then D-block per group, ping-ponging PE and
    DVE at ~50% each).
  - Score dots read qkb straight from PSUM on DVE (a bf16 staging drain
    buys nothing: the 3-op stt has no DVE fast modes).
  - Softmax normalizer r: ONE matmul over a densely packed weight bank
    (v1 used 32 single-column matmuls).
  - E is batched over all 64 classes (v2's per-half E doubled its cost).
  - F consumers split across engines: 1/4 DVE-fused from PSUM, 1/2
    ACT-drain + DVE 2x add, 1/4 ACT-drain + GPSIMD add; res tiles DMA
    out per pair so the output stream rides under F.
"""

import numpy as np

N_CLASSES = 512
K_SHOTS = 64
D = 1024
NK = N_CLASSES * K_SHOTS
N_CORES = 8
C_LOC = N_CLASSES // N_CORES          # 64 classes per core
R_LOC = NK // N_CORES                 # 4096 support rows per core
P = 128
TILES = R_LOC // P                    # 32 row-tiles of 128
DCH = D // P                          # 8 contraction chunks
GSZ = 8                               # tiles per softmax group
GROUPS = TILES // GSZ                 # 4
INV_SQRT_D = 1.0 / float(np.sqrt(D))
WPAD = 66                             # w_all per-tile column pitch (64 + 2)

USE_FP8_B = False                     # fp8 B: 1.9e-2 rel err, too close to gate

_NC_CACHE = None


def _build_nc():
    import concourse.bacc as bacc
    import concourse.tile as tile
    import concourse.mybir as mybir
    from concourse.masks import make_identity

    f32 = mybir.dt.float32
    bf16 = mybir.dt.bfloat16
    fp8 = mybir.dt.float8e4
    ADD = mybir.AluOpType.add
    MUL = mybir.AluOpType.mult
    EXP = mybir.ActivationFunctionType.Exp
    CPY = mybir.ActivationFunctionType.Copy
    DROW = mybir.MatmulPerfMode.DoubleRow

    nc = bacc.Bacc("TRN2", target_bir_lowering=False, debug=False,
                   num_devices=N_CORES)

    sup_d = nc.dram_tensor("sup", [R_LOC, D], bf16, kind="ExternalInput").ap()
    entt_d = nc.dram_tensor("entt", [D, C_LOC], bf16, kind="ExternalInput").ap()
    if USE_FP8_B:
        # ind8[p, i, r] = 1 iff label_loc[r] == 32*i + p  (2 k-tiles of 32)
        ind_d = nc.dram_tensor("ind", [C_LOC // 2, 2 * R_LOC], fp8,
                               kind="ExternalInput").ap()
    else:
        ind_d = nc.dram_tensor("ind", [C_LOC, R_LOC], bf16,
                               kind="ExternalInput").ap()
    # F's broadcast always needs the bf16 indicator
    indf_d = nc.dram_tensor("indf", [C_LOC, R_LOC], bf16,
                            kind="ExternalInput").ap()
    wqk_d = nc.dram_tensor("wqk", [D, D], bf16, kind="ExternalInput").ap()
    wvo_d = nc.dram_tensor("wvo", [D, D], bf16, kind="ExternalInput").ap()
    bqk_d = nc.dram_tensor("bqk", [1, D], bf16, kind="ExternalInput").ap()
    bvo_d = nc.dram_tensor("bvo", [1, D], bf16, kind="ExternalInput").ap()
    res_d = nc.dram_tensor("res", [R_LOC, D], bf16, kind="ExternalOutput").ap()

    with tile.TileContext(nc) as tc:
        with (
            tc.tile_pool(name="const", bufs=1) as const,
            tc.tile_pool(name="sbB", bufs=2) as sbB,
            tc.tile_pool(name="psQ", bufs=2, space="PSUM") as psQ,
            tc.tile_pool(name="psP", bufs=1, space="PSUM") as psP,
            tc.tile_pool(name="psS", bufs=1, space="PSUM") as psS,
            tc.tile_pool(name="psT", bufs=1, space="PSUM") as psT,
        ):
            idf = const.tile([P, P], f32)
            make_identity(nc, idf)
            idb = const.tile([P, P], bf16)
            nc.scalar.copy(out=idb, in_=idf)
            ones_b = const.tile([1, C_LOC], bf16)
            nc.vector.memset(ones_b, 1.0)
            ones_col = const.tile([P, 1], bf16)
            nc.vector.memset(ones_col, 1.0)

            entt_sb = const.tile([P, DCH * C_LOC], bf16)
            if USE_FP8_B:
                ind_sb = const.tile([C_LOC // 2, 2 * R_LOC], fp8)
                qk8_sb = const.tile([C_LOC // 2, 2 * D], fp8)
            else:
                ind_sb = const.tile([C_LOC, R_LOC], bf16)
                qk_sb = const.tile([C_LOC, D], bf16)
            indf_sb = const.tile([C_LOC, R_LOC], bf16)
            wqk_sb = const.tile([P, DCH * D], bf16)
            wvo_sb = const.tile([P, DCH * D], bf16)
            bqk_sb = const.tile([1, D], bf16)
            bvo_sb = const.tile([1, D], bf16)
            out_sb = const.tile([C_LOC, D], bf16)
            pooled_sb = const.tile([C_LOC, D], bf16)
            pooledt_sb = const.tile([P, DCH * C_LOC], bf16)
            sup_all = const.tile([P, TILES * D], bf16)
            ri_sb = const.tile([C_LOC, 1], f32)
            # padded softmax-weight lhsT bank: tile t's two columns live at
            # WPAD*t (+1); its D-matmul lhsT window is [64t, 64t+64) -- only
            # tile t's own pair lands inside its window.
            w_all = const.tile([P, WPAD * TILES], bf16)
            nc.vector.memset(w_all, 0.0)
            # densely packed copy (col 2t, 2t+1) for the single normalizer
            # matmul r = w_r^T @ ones.
            w_r = const.tile([P, C_LOC], bf16)
            nc.vector.memset(w_r, 0.0)

            # ---------------- input DMAs (one FIFO queue, issue order =
            # arrival order): consts -> wqk -> ind -> sup -> wvo ------------
            nc.sync.dma_start(out=bqk_sb, in_=bqk_d)
            nc.sync.dma_start(out=bvo_sb, in_=bvo_d)
            nc.sync.dma_start(
                out=entt_sb.rearrange("p (ch c) -> p ch c", ch=DCH),
                in_=entt_d.rearrange("(ch p) c -> p ch c", p=P),
            )
            wqk_v = wqk_sb.rearrange("p (ch d) -> p ch d", ch=DCH)
            wqkd_v = wqk_d.rearrange("(ch p) d -> p ch d", p=P)
            for h4 in range(4):
                nc.sync.dma_start(out=wqk_v[:, 2 * h4:2 * h4 + 2, :],
                                  in_=wqkd_v[:, 2 * h4:2 * h4 + 2, :])
            nc.sync.dma_start(out=ind_sb, in_=ind_d)
            nc.sync.dma_start(out=indf_sb, in_=indf_d)
            sup_v = sup_all.rearrange("p (t d) -> p t d", d=D)
            supd_v = sup_d.rearrange("(t p) d -> p t d", p=P)
            for k in range(16):
                nc.sync.dma_start(out=sup_v[:, 2 * k:2 * k + 2, :],
                                  in_=supd_v[:, 2 * k:2 * k + 2, :])
            wvo_v = wvo_sb.rearrange("p (ch d) -> p ch d", ch=DCH)
            wvod_v = wvo_d.rearrange("(ch p) d -> p ch d", p=P)
            for h4 in range(4):
                nc.sync.dma_start(out=wvo_v[:, 2 * h4:2 * h4 + 2, :],
                                  in_=wvod_v[:, 2 * h4:2 * h4 + 2, :])

            # ---------------- PE warmup (short: p-state ramp) --------------
            with nc.named_scope("warmup"):
                for _ in range(4):
                    w_ps = psQ.tile([P, D], f32, tag="ring")
                    nc.tensor.transpose(w_ps[:, 0:P], idf, idf)

            # ---------------- Phase A: Qk = entT.T @ Wqk + bqk -------------
            with nc.named_scope("phaseA"):
                if USE_FP8_B:
                    # two 32-class chains so both row-halves of Qk land on
                    # partitions 0-31 (DoubleRow k-tiles share partitions)
                    for half in range(2):
                        q_ps = psQ.tile([P, D], f32, tag="ring")
                        for ch in range(DCH):
                            for nh in range(2):
                                nc.tensor.matmul(
                                    q_ps[0:32, nh * 512:(nh + 1) * 512],
                                    entt_sb[:, ch * C_LOC + 32 * half:
                                            ch * C_LOC + 32 * half + 32],
                                    wqk_sb[:, ch * D + nh * 512:
                                           ch * D + (nh + 1) * 512],
                                    start=(ch == 0), stop=False,
                                )
                        for nh in range(2):
                            nc.tensor.matmul(
                                q_ps[0:32, nh * 512:(nh + 1) * 512],
                                ones_b[0:1, 0:32],
                                bqk_sb[0:1, nh * 512:(nh + 1) * 512],
                                start=False, stop=True,
                            )
                        nc.scalar.copy(
                            out=qk8_sb[:, half * D:(half + 1) * D],
                            in_=q_ps[0:32, :])
                else:
                    q_ps = psQ.tile([P, D], f32, tag="ring")
                    for ch in range(DCH):
                        for nh in range(2):
                            nc.tensor.matmul(
                                q_ps[0:C_LOC, nh * 512:(nh + 1) * 512],
                                entt_sb[:, ch * C_LOC:(ch + 1) * C_LOC],
                                wqk_sb[:, ch * D + nh * 512:
                                       ch * D + (nh + 1) * 512],
                                start=(ch == 0), stop=False,
                            )
                    for nh in range(2):
                        nc.tensor.matmul(
                            q_ps[0:C_LOC, nh * 512:(nh + 1) * 512],
                            ones_b, bqk_sb[0:1, nh * 512:(nh + 1) * 512],
                            start=False, stop=True,
                        )
                    nc.scalar.copy(out=qk_sb, in_=q_ps[0:C_LOC, :])

            res_v = res_d.rearrange("(t p) d -> p t d", p=P)
            prod = sbB.tile([P, D], bf16, tag="prod", bufs=1)
            if USE_FP8_B:
                ind_v = ind_sb.rearrange("p (i r) -> p i r", i=2)
                qk8_v = qk8_sb.rearrange("p (i d) -> p i d", i=2)

            def b_tile(t, s8, j):
                """scores for tile t -> s8[:, j] (exp/normalize later)."""
                qkb = psQ.tile([P, D], f32, tag="ring")
                if USE_FP8_B:
                    for nh in range(2):
                        nc.tensor.matmul(
                            qkb[:, nh * 512:(nh + 1) * 512],
                            ind_v[:, :, t * P:(t + 1) * P],
                            qk8_v[:, :, nh * 512:(nh + 1) * 512],
                            start=True, stop=True,
                            perf_mode=DROW,
                        )
                else:
                    for nh in range(2):
                        nc.tensor.matmul(
                            qkb[:, nh * 512:(nh + 1) * 512],
                            ind_sb[:, t * P:(t + 1) * P],
                            qk_sb[:, nh * 512:(nh + 1) * 512],
                            start=True, stop=True,
                        )
                if t % 3 == 2:
                    # off-DVE dot: ACT drains+scales, GPSIMD multiplies,
                    # DVE only does the cheap 4x-mode accumulate pass
                    qkb_sb = sbB.tile([P, D], bf16, tag="qkb_sb", bufs=2)
                    nc.scalar.activation(out=qkb_sb, in_=qkb, func=CPY,
                                         scale=INV_SQRT_D)
                    pg = sbB.tile([P, D], bf16, tag="prodg", bufs=2)
                    nc.gpsimd.tensor_tensor(
                        out=pg, in0=qkb_sb,
                        in1=sup_all[:, t * D:(t + 1) * D], op=MUL)
                    nc.vector.tensor_scalar(
                        out=pg, in0=pg, scalar1=1.0, scalar2=0.0,
                        op0=MUL, op1=ADD, accum_out=s8[:, j:j + 1])
                else:
                    # DVE reads the PSUM qkb directly (no staging drain)
                    nc.vector.scalar_tensor_tensor(
                        out=prod, in0=qkb, scalar=INV_SQRT_D,
                        in1=sup_all[:, t * D:(t + 1) * D],
                        op0=MUL, op1=MUL, accum_out=s8[:, j:j + 1])

            def c_group(g, s8):
                """exp + scatter the weight pairs into w_all / w_r."""
                e8 = sbB.tile([P, GSZ], bf16, tag="e8", bufs=2)
                nc.scalar.activation(out=e8, in_=s8, func=EXP)
                b_all = WPAD * GSZ * g
                b_r = 2 * GSZ * g
                nc.vector.tensor_copy(
                    out=w_all[0:K_SHOTS, b_all:b_all + WPAD * GSZ:WPAD],
                    in_=e8[0:K_SHOTS, :])
                nc.vector.tensor_copy(
                    out=w_all[K_SHOTS:P,
                              b_all + 1:b_all + WPAD * (GSZ - 1) + 2:WPAD],
                    in_=e8[K_SHOTS:P, :])
                nc.vector.tensor_copy(
                    out=w_r[0:K_SHOTS, b_r:b_r + 2 * GSZ:2],
                    in_=e8[0:K_SHOTS, :])
                nc.vector.tensor_copy(
                    out=w_r[K_SHOTS:P, b_r + 1:b_r + 2 * GSZ:2],
                    in_=e8[K_SHOTS:P, :])

            def d_tile(t, pooled_ps):
                """pooled += w_tile.T @ sup_tile (chained accumulation)."""
                for nh in range(2):
                    nc.tensor.matmul(
                        pooled_ps[:, nh * 512:(nh + 1) * 512],
                        w_all[:, 64 * t:64 * t + 64],
                        sup_all[:, t * D + nh * 512:t * D + (nh + 1) * 512],
                        start=(t == 0), stop=(t == TILES - 1),
                    )

            # ------------- B/C/D software-pipelined per tile ---------------
            # PE order: [B(g,0), D(g-1,0), B(g,1), D(g-1,1), ...] so the PE
            # keeps feeding DVE fresh qkb tiles while it chews D matmuls.
            pooled_ps = psP.tile([C_LOC, D], f32)
            r_ps = psS.tile([C_LOC, 1], f32)
            s8s = []
            with nc.named_scope("phaseBCD"):
                for g in range(GROUPS):
                    s8 = sbB.tile([P, GSZ], f32, tag="s8", bufs=2)
                    s8s.append(s8)
                    for j in range(GSZ):
                        b_tile(g * GSZ + j, s8, j)
                        if g > 0:
                            d_tile((g - 1) * GSZ + j, pooled_ps)
                    c_group(g, s8)
                for j in range(GSZ):
                    d_tile((GROUPS - 1) * GSZ + j, pooled_ps)
                nc.tensor.matmul(r_ps, w_r, ones_col, start=True, stop=True)

            # ---------------- Phase E: OUT = (pooled/r) @ Wvo + bvo --------
            with nc.named_scope("phaseE"):
                nc.vector.reciprocal(ri_sb, r_ps)
                nc.scalar.activation(out=pooled_sb, in_=pooled_ps,
                                     func=CPY, scale=ri_sb[:, 0:1])
                for ch in range(DCH):
                    tp = psT.tile([P, C_LOC], bf16, tag="tp")
                    nc.tensor.transpose(
                        tp, pooled_sb[:, ch * P:(ch + 1) * P],
                        idb[0:C_LOC, 0:C_LOC],
                    )
                    nc.scalar.copy(
                        out=pooledt_sb[:, ch * C_LOC:(ch + 1) * C_LOC],
                        in_=tp,
                    )
                o_ps = psQ.tile([P, D], f32, tag="ring")
                for ch in range(DCH):
                    for nh in range(2):
                        nc.tensor.matmul(
                            o_ps[0:C_LOC, nh * 512:(nh + 1) * 512],
                            pooledt_sb[:, ch * C_LOC:(ch + 1) * C_LOC],
                            wvo_sb[:, ch * D + nh * 512:ch * D + (nh + 1) * 512],
                            start=(ch == 0), stop=False,
                        )
                for nh in range(2):
                    nc.tensor.matmul(
                        o_ps[0:C_LOC, nh * 512:(nh + 1) * 512],
                        ones_b, bvo_sb[0:1, nh * 512:(nh + 1) * 512],
                        start=False, stop=True,
                    )
                nc.scalar.copy(out=out_sb, in_=o_ps[0:C_LOC, :])

            # ---------------- Phase F: res = sup + OUT[class(row)] ---------
            with nc.named_scope("phaseF"):
                for t in range(TILES):
                    ob = psQ.tile([P, D], f32, tag="ring")
                    for nh in range(2):
                        nc.tensor.matmul(
                            ob[:, nh * 512:(nh + 1) * 512],
                            indf_sb[:, t * P:(t + 1) * P],
                            out_sb[:, nh * 512:(nh + 1) * 512],
                            start=True, stop=True,
                        )
                    st = sup_all[:, t * D:(t + 1) * D]
                    if t % 4 == 0:
                        # DVE adds straight from PSUM (fused drain+add)
                        nc.vector.scalar_tensor_tensor(
                            out=st, in0=ob, scalar=1.0, in1=st,
                            op0=MUL, op1=ADD)
                    else:
                        ob_sb = sbB.tile([P, D], bf16, tag="ob_sb", bufs=2)
                        nc.scalar.copy(out=ob_sb, in_=ob)
                        if t % 2 == 1:
                            nc.vector.tensor_tensor(out=st, in0=st, in1=ob_sb,
                                                    op=ADD)
                        else:
                            nc.gpsimd.tensor_tensor(out=st, in0=st, in1=ob_sb,
                                                    op=ADD)
                    if t % 2 == 1:
                        nc.sync.dma_start(
                            out=res_v[:, t - 1:t + 1, :],
                            in_=sup_v[:, t - 1:t + 1, :],
                        )

    nc.compile()
    return nc


def _get_nc():
    global _NC_CACHE
    if _NC_CACHE is None:
        _NC_CACHE = _build_nc()
    return _NC_CACHE


def _prep_in_maps(support_features, entity_vectors, support_labels,
                  Wq, bq, Wk, bk, Wv, bv, Wo, bo):
    from ml_dtypes import bfloat16, float8_e4m3fn

    sup = np.asarray(support_features, dtype=np.float32)
    ent = np.asarray(entity_vectors, dtype=np.float32)
    labels = np.asarray(support_labels, dtype=np.int32)
    wq = np.asarray(Wq, dtype=np.float32)
    wk = np.asarray(Wk, dtype=np.float32)
    wv = np.asarray(Wv, dtype=np.float32)
    wo = np.asarray(Wo, dtype=np.float32)
    bq_ = np.asarray(bq, dtype=np.float32).reshape(1, D)
    bv_ = np.asarray(bv, dtype=np.float32).reshape(1, D)
    bo_ = np.asarray(bo, dtype=np.float32).reshape(1, D)
    # bk is dropped: it adds a per-class constant to each softmax row.

    # weights-only folding (reparameterization; activation math is on-device)
    wqk = np.ascontiguousarray(wq.T @ wk).astype(bfloat16)
    wvo = np.ascontiguousarray(wv.T @ wo.T).astype(bfloat16)
    bqk = (bq_ @ wk).astype(bfloat16)
    bvo = (bv_ @ wo.T + bo_).astype(bfloat16)

    expected = np.arange(NK, dtype=np.int32) // K_SHOTS
    assert np.array_equal(labels, expected), (
        "kernel assumes exactly K_SHOTS contiguous samples per class "
        "(labels == arange(NK)//K_SHOTS)"
    )

    sup_bf = sup.astype(bfloat16)
    in_maps = []
    for c in range(N_CORES):
        lab_loc = labels[c * R_LOC:(c + 1) * R_LOC] - c * C_LOC
        indf = (lab_loc[None, :] ==
                np.arange(C_LOC, dtype=np.int32)[:, None]).astype(bfloat16)
        if USE_FP8_B:
            # [32, 2, R_LOC]: k-tile i holds classes 32i..32i+31
            ind8 = (lab_loc[None, None, :] ==
                    (np.arange(C_LOC, dtype=np.int32)
                     .reshape(2, 32).transpose(1, 0)[:, :, None])
                    ).astype(float8_e4m3fn)
            ind = np.ascontiguousarray(ind8.reshape(32, 2 * R_LOC))
        else:
            ind = indf
        in_maps.append({
            "sup": np.ascontiguousarray(sup_bf[c * R_LOC:(c + 1) * R_LOC]),
            "entt": np.ascontiguousarray(
                ent[c * C_LOC:(c + 1) * C_LOC].T).astype(bfloat16),
            "ind": np.ascontiguousarray(ind),
            "indf": np.ascontiguousarray(indf),
            "wqk": wqk, "wvo": wvo, "bqk": bqk, "bvo": bvo,
        })
    return in_maps


def _run(in_maps, **kwargs):
    from concourse.bass_utils import run_bass_kernel_spmd
    nc = _get_nc()
    return run_bass_kernel_spmd(nc, in_maps, core_ids=list(range(N_CORES)),
                                **kwargs)


def kernel(support_features, entity_vectors, support_labels,
           Wq, bq, Wk, bk, Wv, bv, Wo, bo):
    in_maps = _prep_in_maps(support_features, entity_vectors, support_labels,
                            Wq, bq, Wk, bk, Wv, bv, Wo, bo)
    r = _run(in_maps)
    return np.concatenate(
        [np.asarray(r.results[c]["res"], dtype=np.float32)
         for c in range(N_CORES)], axis=0)


# revision 9
# speedup vs baseline: 1.1762x; 1.0497x over previous
"""EntityGuidedCrossAttention TRN2 kernel (8 NeuronCores, data-parallel over classes).

Math restructure (exact): labels are contiguous per class, so attention is
block-diagonal.  With folded weights (host-side, weights-only algebra):
    Wqk = Wq^T Wk,  bqk = bq Wk          ->  Qk = ent @ Wqk + bqk
    Wvo = Wv^T Wo^T, bvo = bv Wo^T + bo  ->  OUT = pooled @ Wvo + bvo
    score[c,k] = Qk[c] . sup[c*K+k] / sqrt(D)   (bk is softmax-shift-invariant)
    pooled[c]  = sum_k softmax_w[c,k] * sup[c*K+k]
    res        = sup + OUT[class(row)]

v3 (evidence from v1/v2 traces): PE is the wall -- the HW duty-throttles
the tensor engine to ~1.1-1.2GHz effective whenever it is continuously
busy, so PE cycles are ~2x their nominal cost.  Changes:
  - B's onehot broadcast (ind.T @ Qk) runs as an fp8e4 DoubleRow matmul
    (0.5 cycles/row): the 0/1 indicator is exact in fp8, Qk is quantized
    to fp8 (the induced ~3% logit jitter averages out through softmax ->
    ~6e-3 final rel err on top of 5e-3 bf16 base; gate is 2e-2).  A is
    computed as two 32-class chains so both Qk row-halves land on
    partitions 0-31, which DoubleRow's 2-k-tile layout requires.
  - D(g-1) matmuls are interleaved between B(g) matmuls in PE program
    order (v1/v2 ran B-block then D-block per group, ping-ponging PE and
    DVE at ~50% each).
  - Score dots read qkb straight from PSUM on DVE (a bf16 staging drain
    buys nothing: the 3-op stt has no DVE fast modes).
  - Softmax normalizer r: ONE matmul over a densely packed weight bank
    (v1 used 32 single-column matmuls).
  - E is batched over all 64 classes (v2's per-half E doubled its cost).
  - F consumers split across engines: 1/4 DVE-fused from PSUM, 1/2
    ACT-drain + DVE 2x add, 1/4 ACT-drain + GPSIMD add; res tiles DMA
    out per pair so the output stream rides under F.
"""

import numpy as np

N_CLASSES = 512
K_SHOTS = 64
D = 1024
NK = N_CLASSES * K_SHOTS
N_CORES = 8
C_LOC = N_CLASSES // N_CORES          # 64 classes per core
R_LOC = NK // N_CORES                 # 4096 support rows per core
P = 128
TILES = R_LOC // P                    # 32 row-tiles of 128
DCH = D // P                          # 8 contraction chunks
GSZ = 8                               # tiles per softmax group
GROUPS = TILES // GSZ                 # 4
INV_SQRT_D = 1.0 / float(np.sqrt(D))
WPAD = 66                             # w_all per-tile column pitch (64 + 2)

USE_FP8_B = False                     # fp8 B: 1.9e-2 rel err, too close to gate

_NC_CACHE = None


def _build_nc():
    import concourse.bacc as bacc
    import concourse.tile as tile
    import concourse.mybir as mybir
    from concourse.masks import make_identity

    f32 = mybir.dt.float32
    bf16 = mybir.dt.bfloat16
    fp8 = mybir.dt.float8e4
    ADD = mybir.AluOpType.add
    MUL = mybir.AluOpType.mult
    EXP = mybir.ActivationFunctionType.Exp
    CPY = mybir.ActivationFunctionType.Copy
    DROW = mybir.MatmulPerfMode.DoubleRow

    nc = bacc.Bacc("TRN2", target_bir_lowering=False, debug=False,
                   num_devices=N_CORES)

    sup_d = nc.dram_tensor("sup", [R_LOC, D], bf16, kind="ExternalInput").ap()
    entt_d = nc.dram_tensor("entt", [D, C_LOC], bf16, kind="ExternalInput").ap()
    if USE_FP8_B:
        # ind8[p, i, r] = 1 iff label_loc[r] == 32*i + p  (2 k-tiles of 32)
        ind_d = nc.dram_tensor("ind", [C_LOC // 2, 2 * R_LOC], fp8,
                               kind="ExternalInput").ap()
    else:
        ind_d = nc.dram_tensor("ind", [C_LOC, R_LOC], bf16,
                               kind="ExternalInput").ap()
    # F's broadcast always needs the bf16 indicator
    indf_d = nc.dram_tensor("indf", [C_LOC, R_LOC], bf16,
                            kind="ExternalInput").ap()
    wqk_d = nc.dram_tensor("wqk", [D, D], bf16, kind="ExternalInput").ap()
    wvo_d = nc.dram_tensor("wvo", [D, D], bf16, kind="ExternalInput").ap()
    bqk_d = nc.dram_tensor("bqk", [1, D], bf16, kind="ExternalInput").ap()
    bvo_d = nc.dram_tensor("bvo", [1, D], bf16, kind="ExternalInput").ap()
    res_d = nc.dram_tensor("res", [R_LOC, D], bf16, kind="ExternalOutput").ap()

    with tile.TileContext(nc) as tc:
        with (
            tc.tile_pool(name="const", bufs=1) as const,
            tc.tile_pool(name="sbB", bufs=2) as sbB,
            tc.tile_pool(name="psQ", bufs=2, space="PSUM") as psQ,
            tc.tile_pool(name="psP", bufs=1, space="PSUM") as psP,
            tc.tile_pool(name="psT", bufs=2, space="PSUM") as psT,
        ):
            idf = const.tile([P, P], f32)
            make_identity(nc, idf)
            idb = const.tile([P, P], bf16)
            nc.scalar.copy(out=idb, in_=idf)
            ones_b = const.tile([1, C_LOC], bf16)
            nc.vector.memset(ones_b, 1.0)
            ones_col = const.tile([P, 1], bf16)
            nc.vector.memset(ones_col, 1.0)

            entt_sb = const.tile([P, DCH * C_LOC], bf16)
            if USE_FP8_B:
                ind_sb = const.tile([C_LOC // 2, 2 * R_LOC], fp8)
                qk8_sb = const.tile([C_LOC // 2, 2 * D], fp8)
            else:
                ind_sb = const.tile([C_LOC, R_LOC], bf16)
                qk_sb = const.tile([C_LOC, D], bf16)
            indf_sb = const.tile([C_LOC, R_LOC], bf16)
            wqk_sb = const.tile([P, DCH * D], bf16)
            wvo_sb = const.tile([P, DCH * D], bf16)
            bqk_sb = const.tile([1, D], bf16)
            bvo_sb = const.tile([1, D], bf16)
            out_sb = const.tile([C_LOC, D], bf16)
            pooled_sb = const.tile([C_LOC, D], bf16)
            pooledt_sb = const.tile([P, DCH * C_LOC], bf16)
            sup_all = const.tile([P, TILES * D], bf16)
            ri_sb = const.tile([C_LOC, 1], f32)
            # padded softmax-weight lhsT bank: tile t's two columns live at
            # WPAD*t (+1); its D-matmul lhsT window is [64t, 64t+64) -- only
            # tile t's own pair lands inside its window.
            w_all = const.tile([P, WPAD * TILES], bf16)
            nc.vector.memset(w_all, 0.0)
            # densely packed copy (col 2t, 2t+1) for the single normalizer
            # matmul r = w_r^T @ ones.
            w_r = const.tile([P, C_LOC], bf16)
            nc.vector.memset(w_r, 0.0)

            # ---------------- input DMAs (one FIFO queue, issue order =
            # arrival order): consts -> wqk -> ind -> sup -> wvo ------------
            nc.sync.dma_start(out=bqk_sb, in_=bqk_d)
            nc.sync.dma_start(out=bvo_sb, in_=bvo_d)
            nc.sync.dma_start(
                out=entt_sb.rearrange("p (ch c) -> p ch c", ch=DCH),
                in_=entt_d.rearrange("(ch p) c -> p ch c", p=P),
            )
            wqk_v = wqk_sb.rearrange("p (ch d) -> p ch d", ch=DCH)
            wqkd_v = wqk_d.rearrange("(ch p) d -> p ch d", p=P)
            for h4 in range(4):
                nc.sync.dma_start(out=wqk_v[:, 2 * h4:2 * h4 + 2, :],
                                  in_=wqkd_v[:, 2 * h4:2 * h4 + 2, :])
            nc.sync.dma_start(out=ind_sb, in_=ind_d)
            nc.sync.dma_start(out=indf_sb, in_=indf_d)
            sup_v = sup_all.rearrange("p (t d) -> p t d", d=D)
            supd_v = sup_d.rearrange("(t p) d -> p t d", p=P)
            for k in range(16):
                nc.sync.dma_start(out=sup_v[:, 2 * k:2 * k + 2, :],
                                  in_=supd_v[:, 2 * k:2 * k + 2, :])
            wvo_v = wvo_sb.rearrange("p (ch d) -> p ch d", ch=DCH)
            wvod_v = wvo_d.rearrange("(ch p) d -> p ch d", p=P)
            for h4 in range(4):
                nc.sync.dma_start(out=wvo_v[:, 2 * h4:2 * h4 + 2, :],
                                  in_=wvod_v[:, 2 * h4:2 * h4 + 2, :])

            # ---------------- PE warmup (short: p-state ramp) --------------
            with nc.named_scope("warmup"):
                for _ in range(4):
                    w_ps = psQ.tile([P, D], f32, tag="ring")
                    nc.tensor.transpose(w_ps[:, 0:P], idf, idf)

            # ---------------- Phase A: Qk = entT.T @ Wqk + bqk -------------
            with nc.named_scope("phaseA"):
                if USE_FP8_B:
                    # two 32-class chains so both row-halves of Qk land on
                    # partitions 0-31 (DoubleRow k-tiles share partitions)
                    for half in range(2):
                        q_ps = psQ.tile([P, D], f32, tag="ring")
                        for ch in range(DCH):
                            for nh in range(2):
                                nc.tensor.matmul(
                                    q_ps[0:32, nh * 512:(nh + 1) * 512],
                                    entt_sb[:, ch * C_LOC + 32 * half:
                                            ch * C_LOC + 32 * half + 32],
                                    wqk_sb[:, ch * D + nh * 512:
                                           ch * D + (nh + 1) * 512],
                                    start=(ch == 0), stop=False,
                                )
                        for nh in range(2):
                            nc.tensor.matmul(
                                q_ps[0:32, nh * 512:(nh + 1) * 512],
                                ones_b[0:1, 0:32],
                                bqk_sb[0:1, nh * 512:(nh + 1) * 512],
                                start=False, stop=True,
                            )
                        nc.scalar.copy(
                            out=qk8_sb[:, half * D:(half + 1) * D],
                            in_=q_ps[0:32, :])
                else:
                    q_ps = psQ.tile([P, D], f32, tag="ring")
                    for ch in range(DCH):
                        for nh in range(2):
                            nc.tensor.matmul(
                                q_ps[0:C_LOC, nh * 512:(nh + 1) * 512],
                                entt_sb[:, ch * C_LOC:(ch + 1) * C_LOC],
                                wqk_sb[:, ch * D + nh * 512:
                                       ch * D + (nh + 1) * 512],
                                start=(ch == 0), stop=False,
                            )
                    for nh in range(2):
                        nc.tensor.matmul(
                            q_ps[0:C_LOC, nh * 512:(nh + 1) * 512],
                            ones_b, bqk_sb[0:1, nh * 512:(nh + 1) * 512],
                            start=False, stop=True,
                        )
                    nc.scalar.copy(out=qk_sb, in_=q_ps[0:C_LOC, :])

            res_v = res_d.rearrange("(t p) d -> p t d", p=P)
            prod = sbB.tile([P, D], bf16, tag="prod", bufs=1)
            if USE_FP8_B:
                ind_v = ind_sb.rearrange("p (i r) -> p i r", i=2)
                qk8_v = qk8_sb.rearrange("p (i d) -> p i d", i=2)

            def b_tile(t, s8, j):
                """scores for tile t -> s8[:, j] (exp/normalize later)."""
                qkb = psQ.tile([P, D], f32, tag="ring")
                if USE_FP8_B:
                    for nh in range(2):
                        nc.tensor.matmul(
                            qkb[:, nh * 512:(nh + 1) * 512],
                            ind_v[:, :, t * P:(t + 1) * P],
                            qk8_v[:, :, nh * 512:(nh + 1) * 512],
                            start=True, stop=True,
                            perf_mode=DROW,
                        )
                else:
                    for nh in range(2):
                        nc.tensor.matmul(
                            qkb[:, nh * 512:(nh + 1) * 512],
                            ind_sb[:, t * P:(t + 1) * P],
                            qk_sb[:, nh * 512:(nh + 1) * 512],
                            start=True, stop=True,
                        )
                if t % 4 == 3:
                    # ACT drains (folding 1/sqrt(D)); DVE does a bf16 stt
                    qkb_sb = sbB.tile([P, D], bf16, tag="qkb_sb", bufs=2)
                    nc.scalar.activation(out=qkb_sb, in_=qkb, func=CPY,
                                         scale=INV_SQRT_D)
                    nc.vector.scalar_tensor_tensor(
                        out=prod, in0=qkb_sb, scalar=1.0,
                        in1=sup_all[:, t * D:(t + 1) * D],
                        op0=MUL, op1=MUL, accum_out=s8[:, j:j + 1])
                else:
                    # DVE reads the PSUM qkb directly (no staging drain)
                    nc.vector.scalar_tensor_tensor(
                        out=prod, in0=qkb, scalar=INV_SQRT_D,
                        in1=sup_all[:, t * D:(t + 1) * D],
                        op0=MUL, op1=MUL, accum_out=s8[:, j:j + 1])

            def c_group(g, s8):
                """exp + scatter the weight pairs into w_all / w_r."""
                e8 = sbB.tile([P, GSZ], bf16, tag="e8", bufs=2)
                nc.scalar.activation(out=e8, in_=s8, func=EXP)
                b_all = WPAD * GSZ * g
                b_r = 2 * GSZ * g
                nc.vector.tensor_copy(
                    out=w_all[0:K_SHOTS, b_all:b_all + WPAD * GSZ:WPAD],
                    in_=e8[0:K_SHOTS, :])
                nc.vector.tensor_copy(
                    out=w_all[K_SHOTS:P,
                              b_all + 1:b_all + WPAD * (GSZ - 1) + 2:WPAD],
                    in_=e8[K_SHOTS:P, :])
                nc.vector.tensor_copy(
                    out=w_r[0:K_SHOTS, b_r:b_r + 2 * GSZ:2],
                    in_=e8[0:K_SHOTS, :])
                nc.vector.tensor_copy(
                    out=w_r[K_SHOTS:P, b_r + 1:b_r + 2 * GSZ:2],
                    in_=e8[K_SHOTS:P, :])

            def d_tile(t, pooled_ps):
                """pooled += w_tile.T @ sup_tile (chained accumulation)."""
                for nh in range(2):
                    nc.tensor.matmul(
                        pooled_ps[:, nh * 512:(nh + 1) * 512],
                        w_all[:, 64 * t:64 * t + 64],
                        sup_all[:, t * D + nh * 512:t * D + (nh + 1) * 512],
                        start=(t == 0), stop=(t == TILES - 1),
                    )

            # ------------- B/C/D software-pipelined per tile ---------------
            # PE order: [B(g,0), D(g-1,0), B(g,1), D(g-1,1), ...] so the PE
            # keeps feeding DVE fresh qkb tiles while it chews D matmuls.
            pooled_ps = psP.tile([C_LOC, D], f32)
            s8s = []
            with nc.named_scope("phaseBCD"):
                for g in range(GROUPS):
                    s8 = sbB.tile([P, GSZ], f32, tag="s8", bufs=2)
                    s8s.append(s8)
                    for j in range(GSZ):
                        b_tile(g * GSZ + j, s8, j)
                        if g > 0:
                            d_tile((g - 1) * GSZ + j, pooled_ps)
                    c_group(g, s8)
                for j in range(GSZ):
                    d_tile((GROUPS - 1) * GSZ + j, pooled_ps)
                r_slot = psQ.tile([P, D], f32, tag="ring")
                r_ps = r_slot[0:C_LOC, 0:1]
                nc.tensor.matmul(r_ps, w_r, ones_col, start=True, stop=True)

            # ---------------- Phase E: OUT = (pooled/r) @ Wvo + bvo --------
            with nc.named_scope("phaseE"):
                nc.vector.reciprocal(ri_sb, r_ps)
                nc.scalar.activation(out=pooled_sb, in_=pooled_ps,
                                     func=CPY, scale=ri_sb[:, 0:1])
                for ch in range(DCH):
                    tp = psT.tile([P, C_LOC], bf16, tag="tp")
                    nc.tensor.transpose(
                        tp, pooled_sb[:, ch * P:(ch + 1) * P],
                        idb[0:C_LOC, 0:C_LOC],
                    )
                    nc.scalar.copy(
                        out=pooledt_sb[:, ch * C_LOC:(ch + 1) * C_LOC],
                        in_=tp,
                    )
                o_ps = psQ.tile([P, D], f32, tag="ring")
                for ch in range(DCH):
                    for nh in range(2):
                        nc.tensor.matmul(
                            o_ps[0:C_LOC, nh * 512:(nh + 1) * 512],
                            pooledt_sb[:, ch * C_LOC:(ch + 1) * C_LOC],
                            wvo_sb[:, ch * D + nh * 512:ch * D + (nh + 1) * 512],
                            start=(ch == 0), stop=False,
                        )
                for nh in range(2):
                    nc.tensor.matmul(
                        o_ps[0:C_LOC, nh * 512:(nh + 1) * 512],
                        ones_b, bvo_sb[0:1, nh * 512:(nh + 1) * 512],
                        start=False, stop=True,
                    )
                nc.scalar.copy(out=out_sb, in_=o_ps[0:C_LOC, :])

            # ---------------- Phase F: res = sup + OUT[class(row)] ---------
            with nc.named_scope("phaseF"):
                for t in range(TILES):
                    ob = psQ.tile([P, D], f32, tag="ring")
                    for nh in range(2):
                        nc.tensor.matmul(
                            ob[:, nh * 512:(nh + 1) * 512],
                            indf_sb[:, t * P:(t + 1) * P],
                            out_sb[:, nh * 512:(nh + 1) * 512],
                            start=True, stop=True,
                        )
                    st = sup_all[:, t * D:(t + 1) * D]
                    ob_sb = sbB.tile([P, D], bf16, tag="ob_sb", bufs=3)
                    nc.scalar.copy(out=ob_sb, in_=ob)
                    nc.vector.tensor_tensor(out=st, in0=st, in1=ob_sb,
                                            op=ADD)
                    if t % 2 == 1:
                        nc.sync.dma_start(
                            out=res_v[:, t - 1:t + 1, :],
                            in_=sup_v[:, t - 1:t + 1, :],
                        )

    nc.compile()
    return nc


def _get_nc():
    global _NC_CACHE
    if _NC_CACHE is None:
        _NC_CACHE = _build_nc()
    return _NC_CACHE


def _prep_in_maps(support_features, entity_vectors, support_labels,
                  Wq, bq, Wk, bk, Wv, bv, Wo, bo):
    from ml_dtypes import bfloat16, float8_e4m3fn

    sup = np.asarray(support_features, dtype=np.float32)
    ent = np.asarray(entity_vectors, dtype=np.float32)
    labels = np.asarray(support_labels, dtype=np.int32)
    wq = np.asarray(Wq, dtype=np.float32)
    wk = np.asarray(Wk, dtype=np.float32)
    wv = np.asarray(Wv, dtype=np.float32)
    wo = np.asarray(Wo, dtype=np.float32)
    bq_ = np.asarray(bq, dtype=np.float32).reshape(1, D)
    bv_ = np.asarray(bv, dtype=np.float32).reshape(1, D)
    bo_ = np.asarray(bo, dtype=np.float32).reshape(1, D)
    # bk is dropped: it adds a per-class constant to each softmax row.

    # weights-only folding (reparameterization; activation math is on-device)
    wqk = np.ascontiguousarray(wq.T @ wk).astype(bfloat16)
    wvo = np.ascontiguousarray(wv.T @ wo.T).astype(bfloat16)
    bqk = (bq_ @ wk).astype(bfloat16)
    bvo = (bv_ @ wo.T + bo_).astype(bfloat16)

    expected = np.arange(NK, dtype=np.int32) // K_SHOTS
    assert np.array_equal(labels, expected), (
        "kernel assumes exactly K_SHOTS contiguous samples per class "
        "(labels == arange(NK)//K_SHOTS)"
    )

    sup_bf = sup.astype(bfloat16)
    in_maps = []
    for c in range(N_CORES):
        lab_loc = labels[c * R_LOC:(c + 1) * R_LOC] - c * C_LOC
        indf = (lab_loc[None, :] ==
                np.arange(C_LOC, dtype=np.int32)[:, None]).astype(bfloat16)
        if USE_FP8_B:
            # [32, 2, R_LOC]: k-tile i holds classes 32i..32i+31
            ind8 = (lab_loc[None, None, :] ==
                    (np.arange(C_LOC, dtype=np.int32)
                     .reshape(2, 32).transpose(1, 0)[:, :, None])
                    ).astype(float8_e4m3fn)
            ind = np.ascontiguousarray(ind8.reshape(32, 2 * R_LOC))
        else:
            ind = indf
        in_maps.append({
            "sup": np.ascontiguousarray(sup_bf[c * R_LOC:(c + 1) * R_LOC]),
            "entt": np.ascontiguousarray(
                ent[c * C_LOC:(c + 1) * C_LOC].T).astype(bfloat16),
            "ind": np.ascontiguousarray(ind),
            "indf": np.ascontiguousarray(indf),
            "wqk": wqk, "wvo": wvo, "bqk": bqk, "bvo": bvo,
        })
    return in_maps


def _run(in_maps, **kwargs):
    from concourse.bass_utils import run_bass_kernel_spmd
    nc = _get_nc()
    return run_bass_kernel_spmd(nc, in_maps, core_ids=list(range(N_CORES)),
                                **kwargs)


def kernel(support_features, entity_vectors, support_labels,
           Wq, bq, Wk, bk, Wv, bv, Wo, bo):
    in_maps = _prep_in_maps(support_features, entity_vectors, support_labels,
                            Wq, bq, Wk, bk, Wv, bv, Wo, bo)
    r = _run(in_maps)
    return np.concatenate(
        [np.asarray(r.results[c]["res"], dtype=np.float32)
         for c in range(N_CORES)], axis=0)
